# revision 1
# baseline (speedup 1.0000x reference)
"""Trainium2 Bass kernel for nn_BiSPAModule (bidirectional sparse attention).

Full inputs -> full output. Internally: 8-core SPMD.
  - band gather + all input layout prep on host (pure data movement)
  - horizontal attention, MLP+LayerNorm, 3x3 convs, band output: row-sharded
    (32 rows/core, 2-row halo)
  - vertical attention: offset-sharded (16 offsets/core)
  - one AllToAll moves vertical-attention outputs from offset-shard to
    row-shard (with the 2-row halo folded into overlapping 36-row chunks)
  - scatter-add back onto clamped diagonals + triangular mask on host
    (pure data movement; the band holds all nonzeros)

Compute layout on device: activations kept as [channel, token] (channel on
partitions) end-to-end; scores computed transposed [key, query] so softmax
normalization needs no transposes (sums via ones-matmul, broadcast across
partitions by a [128,k]-ones stationary operand).
"""
import numpy as np
import ml_dtypes

import bass_rust
import concourse.bass as bass
import concourse.mybir as mybir
import concourse.tile as tile
from concourse.tile import TileContext
from concourse.vector_clock import ScopedClock
from concourse.masks import make_identity
from concourse.bass_utils import run_bass_kernel_spmd

F32 = mybir.dt.float32
BF16 = mybir.dt.bfloat16
NPBF16 = ml_dtypes.bfloat16
AF = mybir.ActivationFunctionType
ALU = mybir.AluOpType

B, L, C, W, H = 2, 256, 256, 128, 8
D = C // H               # 32 head dim
NC = 8                   # cores
R = L // NC              # 32 rows per core
RH = R + 4               # 36 rows incl halo
O = W // NC              # 16 offsets per core
TOK_H = B * RH * W       # 9216 horizontal tokens per core
TOK_V = B * O * L        # 8192 vertical tokens per core
SCALE = 1.0 / float(np.sqrt(D))
MASK_NEG = -30.0
LN_EPS = 1e-5


# --------------------------------------------------------------------------
# Workarounds: this walrus build rejects instructions with >1 sem wait.
# --------------------------------------------------------------------------
def _patched_drain_and_barrier(self, tick_clock, wait_clock):
    nc = self.nc
    drain_inst = nc.sync.drain()
    wait_clock.add_sem_waits(drain_inst.ins, ScopedClock({None: tick_clock.global_clock}))
    nc.all_engine_barrier()
    assert self.sems is not None
    popped = nc._tile_sem_poison_stack.pop()
    assert popped is self._sem_poison
    nc.clear_and_free_semaphores(list(self.sems.allocated().values()))
    nc.all_engine_barrier()


TileContext._drain_and_barrier = _patched_drain_and_barrier


def split_multi_waits(nc: bass.Bass) -> int:
    """Split any instruction with >1 sem wait into single-wait NoOps followed
    by the original instruction (engines execute in order: equivalent)."""
    n_split = 0
    for f in nc.m.functions:
        for bb in f.blocks:
            insts = bb.instructions
            out = []
            changed = False
            for inst in insts:
                si = inst.sync_info
                if si is not None and len(si.on_wait) > 1:
                    waits = list(si.on_wait)
                    for k, w in enumerate(waits[:-1]):
                        nop = mybir.InstNoOp(name=f"{inst.name}-wsplit{k}", ins=[], outs=[])
                        nop.engine = inst.engine
                        nop.sync_info = bass_rust.SyncInfo(on_wait=[w], on_update=[])
                        out.append(nop)
                    inst.sync_info = bass_rust.SyncInfo(
                        on_wait=[waits[-1]], on_update=list(si.on_update))
                    n_split += 1
                    changed = True
                out.append(inst)
            if changed:
                insts[:] = out
    return n_split


# --------------------------------------------------------------------------
# Device kernel
# --------------------------------------------------------------------------
def build_kernel(use_collective: bool = True, debug: bool = False) -> bass.Bass:
    nc = bass.Bass(num_devices=NC)

    # ---- I/O ----
    xh = nc.dram_tensor("xh", [2, 128, TOK_H], BF16, kind="ExternalInput")
    xv = nc.dram_tensor("xv", [2, 128, TOK_V], BF16, kind="ExternalInput")
    mask_h = nc.dram_tensor("mask_h", [128, RH], F32, kind="ExternalInput")
    mask_v = nc.dram_tensor("mask_v", [128, 2, O], F32, kind="ExternalInput")
    # per-core 0/1 halo masks: cols [sp_top, sp_bot, y1_top, y1_bot]
    emask = nc.dram_tensor("emask", [128, 4], F32, kind="ExternalInput")
    wqkv_h = nc.dram_tensor("wqkv_h", [2, 128, 3 * C], BF16, kind="ExternalInput")
    wqkv_v = nc.dram_tensor("wqkv_v", [2, 128, 3 * C], BF16, kind="ExternalInput")
    bqkv_h = nc.dram_tensor("bqkv_h", [128, 6], F32, kind="ExternalInput")
    bqkv_v = nc.dram_tensor("bqkv_v", [128, 6], F32, kind="ExternalInput")
    wout_h = nc.dram_tensor("wout_h", [2, 128, C], BF16, kind="ExternalInput")
    wout_v = nc.dram_tensor("wout_v", [2, 128, C], BF16, kind="ExternalInput")
    bout_h = nc.dram_tensor("bout_h", [128, 2], F32, kind="ExternalInput")
    wmlp = nc.dram_tensor("wmlp", [4, 128, C], BF16, kind="ExternalInput")
    bmlp = nc.dram_tensor("bmlp", [128, 2], F32, kind="ExternalInput")
    lng = nc.dram_tensor("lng", [128, 2], F32, kind="ExternalInput")
    lnb = nc.dram_tensor("lnb", [128, 2], F32, kind="ExternalInput")
    w1 = nc.dram_tensor("w1", [2, 128, 9, C], BF16, kind="ExternalInput")
    b1 = nc.dram_tensor("b1", [128, 2], F32, kind="ExternalInput")
    w2 = nc.dram_tensor("w2", [2, 128, 9, C], BF16, kind="ExternalInput")
    band = nc.dram_tensor("band", [B, R, W, C], F32, kind="ExternalOutput")
    if debug:
        dbg_zh = nc.dram_tensor("dbg_zh", [2, 128, TOK_H], BF16, kind="ExternalOutput")
        dbg_zv = nc.dram_tensor("dbg_zv", [NC, B, O, RH, C], BF16, kind="ExternalOutput")
        dbg_sp = nc.dram_tensor("dbg_sp", [2, 128, B, RH, W + 2], BF16, kind="ExternalOutput")
        dbg_y1 = nc.dram_tensor("dbg_y1", [2, 128, B, RH - 2, W + 2], BF16, kind="ExternalOutput")
        dbg_qk = nc.dram_tensor("dbg_qk", [128, 4, L], BF16, kind="ExternalOutput")
        dbg_att = nc.dram_tensor("dbg_att", [128, 2, L], BF16, kind="ExternalOutput")
        dbg_raw = nc.dram_tensor("dbg_raw", [4, 128, C], BF16, kind="ExternalOutput")
        dbg_zvt = nc.dram_tensor("dbg_zvt", [128, 2, 4, 128], BF16, kind="ExternalOutput")
        dbg_spre = nc.dram_tensor("dbg_spre", [128, 2, 512], BF16, kind="ExternalOutput")
        dbg_stats = nc.dram_tensor("dbg_stats", [128, 4, 512], F32, kind="ExternalOutput")

    with (
        TileContext(nc) as tc,
        tc.tile_pool(name="consts", bufs=1) as consts,
        tc.tile_pool(name="dram", bufs=1, space="DRAM") as dram,
        tc.tile_pool(name="zh", bufs=1) as zh_pool,
        tc.tile_pool(name="sp", bufs=1) as sp_pool,
        tc.tile_pool(name="y1", bufs=1) as y1_pool,
    ):
        # persistent SBUF tensors
        zh_sb = zh_pool.tile([128, 2, TOK_H], BF16)      # Zh [c2, tokens]
        sp_pad = sp_pool.tile([128, 2, B, RH, W + 2], BF16)
        y1_pad = y1_pool.tile([128, 2, B, RH - 2, W + 2], BF16)

        # constants / weights in SBUF
        ones_sb = consts.tile([128, 128], BF16)
        nc.vector.memset(ones_sb, 1.0)
        ident = consts.tile([128, 128], BF16)
        make_identity(nc, ident)
        eps_sb = consts.tile([128, 1], F32)
        nc.vector.memset(eps_sb, LN_EPS)
        zero_sb = consts.tile([128, 256], BF16)
        nc.vector.memset(zero_sb, 0.0)

        def load_const(h):
            shape = list(h.shape)
            if shape[0] != 128:
                # chunk-major DRAM [k, 128, ...] -> SBUF [128, k, ...]
                assert shape[1] == 128
                t = consts.tile([128, shape[0]] + shape[2:], h.dtype,
                                name=f"c_{h.name}")
                for k in range(shape[0]):
                    nc.sync.dma_start(t[:, k], h[k])
            else:
                t = consts.tile(shape, h.dtype, name=f"c_{h.name}")
                nc.sync.dma_start(t[:], h[:])
            return t

        wqkv_h_sb = load_const(wqkv_h)
        wqkv_v_sb = load_const(wqkv_v)
        bqkv_h_sb = load_const(bqkv_h)
        bqkv_v_sb = load_const(bqkv_v)
        wout_h_sb = load_const(wout_h)
        wout_v_sb = load_const(wout_v)
        bout_h_sb = load_const(bout_h)
        wmlp_sb = load_const(wmlp)
        bmlp_sb = load_const(bmlp)
        lng_sb = load_const(lng)
        lnb_sb = load_const(lnb)
        w1_sb = load_const(w1)
        b1_sb = load_const(b1)
        w2_sb = load_const(w2)
        mask_h_sb = load_const(mask_h)
        mask_v_sb = load_const(mask_v)
        emask_sb = load_const(emask)

        # DRAM intermediates for the collective
        a2a_in = dram.tile([NC, B, O, RH, C], BF16)
        a2a_out = dram.tile([NC, B, O, RH, C], BF16)

        # ============================================================
        # Stage V: vertical attention (offset shard), emits a2a_in
        # ============================================================
        with (
            tc.tile_pool(name="v_sbuf", bufs=3) as vp,
            tc.tile_pool(name="v_small", bufs=4) as vps,
            tc.tile_pool(name="v_psum", bufs=4, space="PSUM") as pp,
            tc.tile_pool(name="v_psum_sc", bufs=4, space="PSUM") as pps,
        ):
            # zero the pad rows of a2a_in chunk 0 (global rows -2,-1) and
            # chunk 7 (global rows 256,257)
            for bb in range(B):
                for k in range(2):
                    nc.sync.dma_start(a2a_in[0, bb, :, k, :], zero_sb[0:O, :])
                    nc.sync.dma_start(a2a_in[NC - 1, bb, :, RH - 2 + k, :],
                                      zero_sb[0:O, :])

            for seq in range(B * O):
                bb, oo = seq // O, seq % O
                tok0 = seq * L
                xt = vp.tile([128, 2, L], BF16, name="xt_v")
                for cc in range(2):
                    nc.sync.dma_start(xt[:, cc, :], xv[cc, :, tok0:tok0 + L])

                # q,k projections -> [f, t] layout (f-chunks: q0,q1,k0,k1)
                qk = vp.tile([128, 4, L], BF16, name="qk_v")
                for fc in range(4):
                    ps = pp.tile([128, L], F32, name="ps_qk", tag="ps")
                    for cc in range(2):
                        nc.tensor.matmul(
                            ps[:], wqkv_v_sb[:, cc, fc * 128:(fc + 1) * 128],
                            xt[:, cc, :], start=(cc == 0), stop=(cc == 1))
                    nc.scalar.activation(qk[:, fc, :], ps[:], AF.Identity,
                                         bias=bqkv_v_sb[:, fc:fc + 1])

                # v projection -> [t, hd] layout
                vsb = vp.tile([128, 2, C], BF16, name="v_v")
                for t2 in range(2):
                    ps = pp.tile([128, C], F32, name="ps_v", tag="ps")
                    for cc in range(2):
                        nc.tensor.matmul(
                            ps[:], xt[:, cc, t2 * 128:(t2 + 1) * 128],
                            wqkv_v_sb[:, cc, 512:768], start=(cc == 0), stop=(cc == 1))
                    nc.vector.tensor_copy(vsb[:, t2, :], ps[:])

                # scores_T = exp(k^T q * scale + mask) per (head, kchunk)
                expt = vp.tile([128, H, 2, L], BF16, name="exp_v")
                for g in range(2):
                    for kc in range(2):
                        psc = [pps.tile([128, L], F32, name="ps_sc", tag="psc") for _ in range(4)]
                        for h4 in range(4):
                            h = 4 * g + h4
                            nc.tensor.matmul(
                                psc[h4][:],
                                qk[32 * h4:32 * h4 + 32, 2 + g, kc * 128:kc * 128 + 128],
                                qk[32 * h4:32 * h4 + 32, g, :],
                                start=True, stop=True,
                                tile_position=(32 * h4, 0))
                        for h4 in range(4):
                            h = 4 * g + h4
                            nc.scalar.activation(
                                expt[:, h, kc, :], psc[h4][:], AF.Exp,
                                bias=mask_v_sb[:, kc, oo:oo + 1], scale=SCALE)

                # sums (broadcast over 32 partitions per head) and av
                att = vps.tile([128, 2, L], BF16, name="att_v")
                for g in range(2):
                    psum_s = pp.tile([128, L], F32, name="ps_sum", tag="ps")
                    for kc in range(2):
                        for h4 in range(4):
                            h = 4 * g + h4
                            nc.tensor.matmul(
                                psum_s[32 * h4:32 * h4 + 32, :],
                                ones_sb[:, :32], expt[:, h, kc, :],
                                start=(kc == 0), stop=(kc == 1),
                                tile_position=(0, 32 * h4))
                    rec = vps.tile([128, L], F32, name="rec_v")
                    nc.vector.reciprocal(rec[:], psum_s[:])
                    psum_a = pp.tile([128, L], F32, name="ps_av", tag="ps")
                    for kc in range(2):
                        for h4 in range(4):
                            h = 4 * g + h4
                            nc.tensor.matmul(
                                psum_a[32 * h4:32 * h4 + 32, :],
                                vsb[:, kc, 32 * h:32 * h + 32], expt[:, h, kc, :],
                                start=(kc == 0), stop=(kc == 1),
                                tile_position=(0, 32 * h4))
                    nc.vector.scalar_tensor_tensor(
                        att[:, g, :], psum_a[:], 1.0, rec[:],
                        op0=ALU.mult, op1=ALU.mult)

                if debug and seq == 0:
                    nc.sync.dma_start(dbg_qk[:], qk[:])
                    nc.sync.dma_start(dbg_att[:], att[:])

                # out-proj V in [t, c2] layout; write overlapped a2a_in chunks
                for t2 in range(2):
                    ps = pp.tile([128, C], F32, name="ps_zv", tag="ps")
                    for g in range(2):
                        nc.tensor.matmul(
                            ps[:], att[:, g, t2 * 128:(t2 + 1) * 128],
                            wout_v_sb[:, g, :], start=(g == 0), stop=(g == 1))
                    zv = vps.tile([128, C], BF16, name="zv_sb")
                    nc.scalar.activation(zv[:], ps[:], AF.Copy)
                    i0 = t2 * 128
                    for j in range(NC):
                        lo = max(i0, 32 * j - 2)
                        hi = min(i0 + 128, 32 * j + RH - 2)
                        if lo >= hi:
                            continue
                        nc.sync.dma_start(
                            a2a_in[j, bb, oo, lo - (32 * j - 2):hi - (32 * j - 2), :],
                            zv[lo - i0:hi - i0, :])

        # ============================================================
        # AllToAll (overlaps with stage H in the schedule)
        # ============================================================
        if use_collective:
            nc.gpsimd.collective_compute(
                "AllToAll", ALU.bypass,
                replica_groups=[list(range(NC))],
                ins=[a2a_in.opt()], outs=[a2a_out.opt()])
        else:
            nc.sync.dma_start(a2a_out[:], a2a_in[:])
        if debug:
            nc.sync.dma_start(dbg_zv[:], a2a_out[:])

        # ============================================================
        # Stage H: horizontal attention (row shard incl halo) -> zh_sb
        # ============================================================
        with (
            tc.tile_pool(name="h_sbuf", bufs=3) as hp,
            tc.tile_pool(name="h_small", bufs=4) as hps,
            tc.tile_pool(name="h_psum", bufs=4, space="PSUM") as pp,
            tc.tile_pool(name="h_psum_sc", bufs=4, space="PSUM") as pps,
        ):
            n_chunks = B * RH // 4          # 18 chunks of 4 rows (512 tokens)
            for ch in range(n_chunks):
                bb, r4 = ch // (RH // 4), ch % (RH // 4)
                tok0 = ch * 512
                xt = hp.tile([128, 2, 512], BF16, name="xt_h")
                for cc in range(2):
                    nc.sync.dma_start(xt[:, cc, :], xh[cc, :, tok0:tok0 + 512])

                qk = hp.tile([128, 4, 512], BF16, name="qk_h")
                for fc in range(4):
                    ps = pp.tile([128, 512], F32, name="ps_qkh", tag="ps")
                    for cc in range(2):
                        nc.tensor.matmul(
                            ps[:], wqkv_h_sb[:, cc, fc * 128:(fc + 1) * 128],
                            xt[:, cc, :], start=(cc == 0), stop=(cc == 1))
                    nc.scalar.activation(qk[:, fc, :], ps[:], AF.Identity,
                                         bias=bqkv_h_sb[:, fc:fc + 1])

                vsb = hp.tile([128, 4, C], BF16, name="v_h")
                for s4 in range(4):
                    ps = pp.tile([128, C], F32, name="ps_vh", tag="ps")
                    for cc in range(2):
                        nc.tensor.matmul(
                            ps[:], xt[:, cc, s4 * 128:(s4 + 1) * 128],
                            wqkv_h_sb[:, cc, 512:768], start=(cc == 0), stop=(cc == 1))
                    nc.vector.tensor_copy(vsb[:, s4, :], ps[:])

                att = hps.tile([128, 2, 512], BF16, name="att_h")
                for s4 in range(4):
                    row = r4 * 4 + s4
                    tsl = slice(s4 * 128, (s4 + 1) * 128)
                    expt = hps.tile([128, H, 128], BF16, name="exp_h")
                    for g in range(2):
                        psc = [pps.tile([128, 128], F32, name="ps_sch", tag="psc") for _ in range(4)]
                        for h4 in range(4):
                            nc.tensor.matmul(
                                psc[h4][:],
                                qk[32 * h4:32 * h4 + 32, 2 + g, tsl],
                                qk[32 * h4:32 * h4 + 32, g, tsl],
                                start=True, stop=True,
                                tile_position=(32 * h4, 0))
                        for h4 in range(4):
                            h = 4 * g + h4
                            nc.scalar.activation(
                                expt[:, h, :], psc[h4][:], AF.Exp,
                                bias=mask_h_sb[:, row:row + 1], scale=SCALE)
                    for g in range(2):
                        psum_s = pp.tile([128, 128], F32, name="ps_sumh", tag="ps")
                        for h4 in range(4):
                            h = 4 * g + h4
                            nc.tensor.matmul(
                                psum_s[32 * h4:32 * h4 + 32, :],
                                ones_sb[:, :32], expt[:, h, :],
                                start=True, stop=True,
                                tile_position=(0, 32 * h4))
                        rec = hps.tile([128, 128], F32, name="rec_h")
                        nc.vector.reciprocal(rec[:], psum_s[:])
                        psum_a = pp.tile([128, 128], F32, name="ps_avh", tag="ps")
                        for h4 in range(4):
                            h = 4 * g + h4
                            nc.tensor.matmul(
                                psum_a[32 * h4:32 * h4 + 32, :],
                                vsb[:, s4, 32 * h:32 * h + 32], expt[:, h, :],
                                start=True, stop=True,
                                tile_position=(0, 32 * h4))
                        nc.vector.scalar_tensor_tensor(
                            att[:, g, tsl], psum_a[:], 1.0, rec[:],
                            op0=ALU.mult, op1=ALU.mult)

                # out-proj H -> zh_sb [c2, tokens]
                for mc in range(2):
                    ps = pp.tile([128, 512], F32, name="ps_zh", tag="ps")
                    for g in range(2):
                        nc.tensor.matmul(
                            ps[:], wout_h_sb[:, g, mc * 128:(mc + 1) * 128],
                            att[:, g, :], start=(g == 0), stop=(g == 1))
                    nc.scalar.activation(zh_sb[:, mc, tok0:tok0 + 512], ps[:],
                                         AF.Identity, bias=bout_h_sb[:, mc:mc + 1])

        if debug:
            for cc in range(2):
                nc.sync.dma_start(dbg_zh[cc], zh_sb[:, cc, :])

        # ============================================================
        # Stage MLP + LayerNorm -> sp_pad
        # ============================================================
        with (
            tc.tile_pool(name="m_sbuf", bufs=3) as mp,
            tc.tile_pool(name="m_f32", bufs=3) as mf,
            tc.tile_pool(name="m_psum", bufs=4, space="PSUM") as pp,
            tc.tile_pool(name="m_psum_t", bufs=4, space="PSUM") as ppt,
        ):
            n_chunks = B * RH // 4
            for ch in range(n_chunks):
                bb, r4 = ch // (RH // 4), ch % (RH // 4)
                tok0 = ch * 512
                # Zv part: gather + transpose to [c2, token]
                zvt = mp.tile([128, 2, 4, 128], BF16, name="zvt")
                for il in range(4):
                    iw = r4 * 4 + il
                    raw = mp.tile([128, C], BF16, name="zv_raw")
                    for s in range(NC):
                        nc.sync.dma_start(raw[s * O:(s + 1) * O, :],
                                          a2a_out[s, bb, :, iw, :])
                    if debug and ch == 0:
                        nc.sync.dma_start(dbg_raw[il], raw[:])
                    for cc in range(2):
                        pst = ppt.tile([128, 128], BF16, name="ps_tr", tag="pst")
                        nc.tensor.transpose(pst[:], raw[:, cc * 128:(cc + 1) * 128],
                                            ident)
                        nc.vector.tensor_copy(zvt[:, cc, il, :], pst[:])

                if debug and ch == 0:
                    nc.sync.dma_start(dbg_zvt[:], zvt[:])
                sp_re = mp.tile([128, 2, 512], BF16, name="sp_re")
                for mc in range(2):
                    ps = pp.tile([128, 512], F32, name="ps_mlp", tag="ps")
                    for kc in range(2):
                        nc.tensor.matmul(
                            ps[:], wmlp_sb[:, kc, mc * 128:(mc + 1) * 128],
                            zh_sb[:, kc, tok0:tok0 + 512],
                            start=(kc == 0), stop=False)
                    for kc in range(2):
                        nc.tensor.matmul(
                            ps[:], wmlp_sb[:, 2 + kc, mc * 128:(mc + 1) * 128],
                            zvt[:, kc, :, :].rearrange("p a b -> p (a b)"),
                            start=False, stop=(kc == 1))
                    nc.scalar.activation(sp_re[:, mc, :], ps[:], AF.Relu,
                                         bias=bmlp_sb[:, mc:mc + 1])

                if debug and ch == 0:
                    nc.sync.dma_start(dbg_spre[:], sp_re[:])
                sq = mp.tile([128, 2, 512], BF16, name="sq")
                nc.vector.tensor_mul(sq[:, 0, :], sp_re[:, 0, :], sp_re[:, 0, :])
                nc.vector.tensor_mul(sq[:, 1, :], sp_re[:, 1, :], sp_re[:, 1, :])
                ps_sum = pp.tile([128, 512], F32, name="ps_lns", tag="ps")
                ps_sq = pp.tile([128, 512], F32, name="ps_lnq", tag="ps")
                for cc in range(2):
                    nc.tensor.matmul(ps_sum[:], ones_sb[:], sp_re[:, cc, :],
                                     start=(cc == 0), stop=(cc == 1))
                for cc in range(2):
                    nc.tensor.matmul(ps_sq[:], ones_sb[:], sq[:, cc, :],
                                     start=(cc == 0), stop=(cc == 1))
                mu = mf.tile([128, 512], F32, name="mu")
                nc.vector.tensor_scalar_mul(mu[:], ps_sum[:], 1.0 / C)
                msq = mf.tile([128, 512], F32, name="msq")
                nc.vector.tensor_scalar_mul(msq[:], ps_sq[:], 1.0 / C)
                var = mf.tile([128, 512], F32, name="var")
                nc.vector.tensor_mul(var[:], mu[:], mu[:])
                nc.vector.tensor_tensor(var[:], msq[:], var[:], ALU.subtract)
                std = mf.tile([128, 512], F32, name="std")
                nc.scalar.activation(std[:], var[:], AF.Sqrt, bias=eps_sb[:])
                rstd = mf.tile([128, 512], F32, name="rstd")
                nc.vector.reciprocal(rstd[:], std[:])
                if debug and ch == 0:
                    nc.sync.dma_start(dbg_stats[:, 0, :], mu[:])
                    nc.sync.dma_start(dbg_stats[:, 1, :], msq[:])
                    nc.sync.dma_start(dbg_stats[:, 2, :], var[:])
                    nc.sync.dma_start(dbg_stats[:, 3, :], rstd[:])
                for cc in range(2):
                    t1 = mf.tile([128, 512], F32, name="t1")
                    nc.vector.tensor_tensor(t1[:], sp_re[:, cc, :], mu[:], ALU.subtract)
                    nc.vector.tensor_mul(t1[:], t1[:], rstd[:])
                    dst = sp_pad[:, cc, bb, r4 * 4:r4 * 4 + 4, 1:W + 1]
                    nc.vector.tensor_scalar(
                        dst, t1.rearrange("p (a b) -> p a b", a=4),
                        scalar1=lng_sb[:, cc:cc + 1], scalar2=lnb_sb[:, cc:cc + 1],
                        op0=ALU.mult, op1=ALU.add)

            # zero pad columns; mask globally-out-of-range halo rows
            nc.vector.memset(sp_pad[:, :, :, :, 0:1], 0.0)
            nc.vector.memset(sp_pad[:, :, :, :, W + 1:W + 2], 0.0)
            for (rows, col) in ((slice(0, 2), 0), (slice(RH - 2, RH), 1)):
                sl = sp_pad[:, :, :, rows, :]
                nc.vector.tensor_scalar_mul(sl, sl, emask_sb[:, col:col + 1])

        if debug:
            for cc in range(2):
                nc.sync.dma_start(dbg_sp[cc], sp_pad[:, cc])

        # ============================================================
        # Stage conv1 -> y1_pad   (34 rows: global r0-1 .. r0+32)
        # ============================================================
        with (
            tc.tile_pool(name="c1_psum", bufs=4, space="PSUM") as pp,
        ):
            row_tiles = [(rt * 4, min(4, (RH - 2) - rt * 4)) for rt in range((RH - 2 + 3) // 4)]
            for bb in range(B):
                for (row0, nr) in row_tiles:
                    for mc in range(2):
                        ps = pp.tile([128, 512], F32, name="ps_c1", tag="ps")[:, :nr * 128]
                        first = True
                        for dy in range(3):
                            for dx in range(3):
                                for cc in range(2):
                                    nc.tensor.matmul(
                                        ps[:],
                                        w1_sb[:, cc, dy * 3 + dx, mc * 128:(mc + 1) * 128],
                                        sp_pad[:, cc, bb, row0 + dy:row0 + dy + nr, dx:dx + 128],
                                        start=first,
                                        stop=(dy == 2 and dx == 2 and cc == 1))
                                    first = False
                        dst = y1_pad[:, mc, bb, row0:row0 + nr, 1:W + 1]
                        nc.scalar.activation(
                            dst, ps.rearrange("p (r x) -> p r x", r=nr),
                            AF.Relu, bias=b1_sb[:, mc:mc + 1])
            nc.vector.memset(y1_pad[:, :, :, :, 0:1], 0.0)
            nc.vector.memset(y1_pad[:, :, :, :, W + 1:W + 2], 0.0)
            for (row, col) in ((0, 2), (RH - 3, 3)):
                sl = y1_pad[:, :, :, row, :]
                nc.vector.tensor_scalar_mul(sl, sl, emask_sb[:, col:col + 1])

        if debug:
            for cc in range(2):
                nc.sync.dma_start(dbg_y1[cc], y1_pad[:, cc])

        # ============================================================
        # Stage conv2 -> band output [t=x, co] per (b, row)
        # ============================================================
        with (
            tc.tile_pool(name="c2_sbuf", bufs=3) as cp,
            tc.tile_pool(name="c2_psum", bufs=4, space="PSUM") as pp,
        ):
            for bb in range(B):
                for z in range(R):
                    ps = pp.tile([128, C], F32, name="ps_c2", tag="ps")
                    first = True
                    for dy in range(3):
                        for dx in range(3):
                            for cc in range(2):
                                nc.tensor.matmul(
                                    ps[:],
                                    y1_pad[:, cc, bb, z + dy, dx:dx + 128],
                                    w2_sb[:, cc, dy * 3 + dx, :],
                                    start=first,
                                    stop=(dy == 2 and dx == 2 and cc == 1))
                                first = False
                    y2 = cp.tile([128, C], F32, name="y2")
                    nc.scalar.activation(y2[:], ps[:], AF.Relu)
                    nc.sync.dma_start(band[bb, z, :, :], y2[:])

    split_multi_waits(nc)
    return nc


# --------------------------------------------------------------------------
# Host side
# --------------------------------------------------------------------------
def _prep_shared(weights):
    """Weight tensors in device layouts (shared across cores)."""
    def qkv_T(w):  # [3C, C] -> [2, 128, 768] (lhsT [c, f])
        t = w.T.astype(NPBF16)                       # [C, 3C]
        return t.reshape(2, 128, 3 * C)

    def col2(v):   # [256] -> [128, 2] f32 (f-chunk columns)
        return np.ascontiguousarray(v.reshape(2, 128).T.astype(np.float32))

    out = {}
    out["wqkv_h"] = qkv_T(weights["h_in_w"])
    out["wqkv_v"] = qkv_T(weights["v_in_w"])
    out["bqkv_h"] = np.ascontiguousarray(
        weights["h_in_b"][:768].reshape(6, 128).T.astype(np.float32))
    out["bqkv_v"] = np.ascontiguousarray(
        weights["v_in_b"][:768].reshape(6, 128).T.astype(np.float32))
    # out-proj: [hd, c2] layout
    out["wout_h"] = weights["h_out_w"].T.astype(NPBF16).reshape(2, 128, C)
    out["wout_v"] = weights["v_out_w"].T.astype(NPBF16).reshape(2, 128, C)
    out["bout_h"] = col2(weights["h_out_b"])
    out["wmlp"] = weights["mlp_w"].T.astype(NPBF16).reshape(4, 128, C)
    out["bmlp"] = col2(weights["mlp_b"])
    out["lng"] = col2(weights["ln_g"])
    out["lnb"] = col2(weights["ln_b"])
    # conv weights: [co, ci, 3, 3] -> [ci, (ky kx), co] -> [2, 128, 9, co]
    for name, key in (("w1", "conv1_w"), ("w2", "conv2_w")):
        w = weights[key].transpose(1, 2, 3, 0).reshape(C, 9, C)   # [ci, tap, co]
        out[name] = w.reshape(2, 128, 9, C).astype(NPBF16)
    out["b1"] = col2(weights["conv1_b"])
    # free-dim biases must be zero (they are, from setup_inputs)
    for k in ("conv2_b",):
        assert np.abs(weights[k]).max() == 0.0, f"{k} must be zero"
    assert np.abs(weights["h_in_b"][512:]).max() == 0.0
    assert np.abs(weights["v_in_b"][512:]).max() == 0.0
    assert np.abs(weights["v_out_b"]).max() == 0.0
    return out


def _prep_core(Sh, j):
    """Per-core activation inputs for core j."""
    r0 = j * R
    rows = np.arange(r0 - 2, r0 + R + 2)
    valid = (rows >= 0) & (rows < L)
    ii = np.arange(L)

    xh_f = np.zeros((B, RH, W, C), np.float32)
    xh_f[:, valid] = Sh[:, rows[valid]]
    # -> [c, (b, i, o)] -> [2, 128, TOK_H]
    xh = xh_f.transpose(3, 0, 1, 2).reshape(C, TOK_H).reshape(2, 128, TOK_H)
    xh = xh.astype(NPBF16)

    o0 = j * O
    xv_f = Sh[:, :, o0:o0 + O, :]                   # [B, L, O, C]
    xv = xv_f.transpose(3, 0, 2, 1).reshape(C, TOK_V).reshape(2, 128, TOK_V)
    xv = xv.astype(NPBF16)

    kb_h = np.where((rows[:, None] + np.arange(W)[None, :]) >= L, MASK_NEG, 0.0)
    kb_h[~valid] = 0.0
    mask_h = np.ascontiguousarray(kb_h.T.astype(np.float32))        # [128, 36]

    kb_v = np.where((ii[:, None] + np.arange(o0, o0 + O)[None, :]) >= L,
                    MASK_NEG, 0.0)                                   # [L, O]
    mask_v = np.ascontiguousarray(
        kb_v.reshape(2, 128, O).transpose(1, 0, 2).astype(np.float32))

    em = np.ones(4, np.float32)
    if j == 0:
        em[0] = 0.0   # sp_top (global rows -2,-1)
        em[2] = 0.0   # y1_top (global row -1)
    if j == NC - 1:
        em[1] = 0.0   # sp_bot
        em[3] = 0.0   # y1_bot
    emask = np.broadcast_to(em[None, :], (128, 4)).astype(np.float32).copy()

    return {"xh": xh, "xv": xv, "mask_h": mask_h, "mask_v": mask_v, "emask": emask}


def _assemble(bands):
    """bands: list of [B, R, W, C] per core -> full [B, L, L, C] output."""
    out = np.zeros((B, L, L, C), np.float32)
    for j in range(NC):
        band = bands[j]
        r0 = j * R
        for z in range(R):
            i = r0 + z
            wv = min(W, L - i)
            out[:, i, i:i + wv, :] = band[:, z, :wv, :]
            if wv < W:
                out[:, i, L - 1, :] += band[:, z, wv:, :].sum(axis=1)
    return out


_NC_CACHE = {}


def get_nc(use_collective=True):
    key = use_collective
    if key not in _NC_CACHE:
        _NC_CACHE[key] = build_kernel(use_collective)
    return _NC_CACHE[key]


def kernel(**inputs) -> np.ndarray:
    inputs = {k: np.asarray(v) for k, v in inputs.items()}
    S = inputs["S"].astype(np.float32)

    ii = np.arange(L)
    idx = np.clip(ii[:, None] + np.arange(W)[None, :], 0, L - 1)
    Sh = S[:, ii[:, None], idx, :]                   # [B, L, W, C]

    shared = _prep_shared(inputs)
    in_maps = []
    for j in range(NC):
        m = dict(shared)
        m.update(_prep_core(Sh, j))
        in_maps.append(m)

    nc = get_nc(use_collective=True)
    res = run_bass_kernel_spmd(nc, in_maps, core_ids=list(range(NC)))
    bands = [res.results[j]["band"] for j in range(NC)]
    return _assemble(bands)


if __name__ == "__main__":
    # quick smoke: random small check against golden.py decomposition
    import reference
    ins = {k: np.asarray(v) for k, v in reference.setup_inputs().items()}
    got = kernel(**ins)
    want = np.asarray(reference.reference(**ins))
    err = np.abs(got - want).max() / np.abs(want).max()
    print(f"kernel vs reference rel err: {err:.3e}")

